# revision 16
# baseline (speedup 1.0000x reference)
"""Trainium2 Bass kernel for DBFLinear:
    y = ((x * s0) @ unpack(bp1).T * s2) @ unpack(bp3).T * s4 + bias

Strategy: data-parallel over batch across 8 cores (weights replicated, no
collectives). Per core: unpack the bit-packed +/-1 weights on device
(DVE bitwise_and + ACT Sign), transpose weight blocks with the DMA xbar,
run both GEMMs weight-stationary (fp16, fp32 PSUM accumulation). scaling0
is folded into the unpacked W1 (+/-s0 is exact in fp16), scaling2 into the
h eviction, scaling4+bias into the y eviction — all per-partition ACT ops.
The device emits y.T per batch shard; the host transposes while unsharding.
"""

import sys

import numpy as np

sys.path.insert(0, "/opt/trn_rl_repo")

import concourse.bass as bass
import concourse.mybir as mybir
import concourse.tile as tile
from concourse.tile import add_dep_helper
from concourse import bacc
from concourse.bass_utils import run_bass_kernel_spmd

N_CORES = 8
B_FULL, IN, MID, OUT = 8192, 4096, 4096, 4096
P = 128
FD = 512  # matmul moving-operand free dim (1 PSUM bank of fp32)
QCH = 1024  # unpack quarter width (weight elements per DVE/ACT op)
N_WARM = 700  # HAM warm-up matmuls


def build_program(b=B_FULL // N_CORES, in_=IN, mid=MID, out=OUT):
    """Build the per-core Bass program. Returns the Bass object."""
    in_k, mid_k, out_k = in_ // P, mid // P, out // P
    nbc = 2  # batch processed as two halves
    fd = b // nbc
    assert fd <= FD, (b, fd)
    uch = min(QCH, in_, mid)

    nc = bacc.Bacc(num_devices=N_CORES)
    x_d = nc.dram_tensor("x", [b, in_], mybir.dt.float16, kind="ExternalInput")
    bp1_d = nc.dram_tensor("bp1", [mid, in_ // 8], mybir.dt.int32, kind="ExternalInput")
    bp3_d = nc.dram_tensor("bp3", [out, mid // 8], mybir.dt.int32, kind="ExternalInput")
    mask_d = nc.dram_tensor("mask", [P, 8], mybir.dt.int32, kind="ExternalInput")
    s0r_d = nc.dram_tensor("s0rep", [P, in_], mybir.dt.float16, kind="ExternalInput")
    s2_d = nc.dram_tensor("s2", [P, mid_k], mybir.dt.float32, kind="ExternalInput")
    s4_d = nc.dram_tensor("s4", [P, out_k], mybir.dt.float32, kind="ExternalInput")
    bias_d = nc.dram_tensor("bias", [P, out_k], mybir.dt.float32, kind="ExternalInput")
    yT_d = nc.dram_tensor("yT", [out, b], mybir.dt.float16, kind="ExternalOutput")

    Act = mybir.ActivationFunctionType

    with tile.TileContext(nc) as tc:
        with (
            tc.tile_pool(name="big", bufs=1) as big,
            tc.tile_pool(name="consts", bufs=1) as consts,
            tc.tile_pool(name="wpipe", bufs=2) as wpipe,
            tc.tile_pool(name="psum", bufs=4, space="PSUM") as psum,
        ):
            mask_t = consts.tile([P, 8], mybir.dt.int32)
            s0r_t = consts.tile([P, in_], mybir.dt.float16)
            s2_t = consts.tile([P, mid_k], mybir.dt.float32)
            s4_t = consts.tile([P, out_k], mybir.dt.float32)
            bias_t = consts.tile([P, out_k], mybir.dt.float32)
            neg_half = consts.tile([P, 1], mybir.dt.float32)
            for t, d in (
                (mask_t, mask_d),
                (s0r_t, s0r_d),
                (s2_t, s2_d),
                (s4_t, s4_d),
                (bias_t, bias_d),
            ):
                nc.sync.dma_start(t[:], d[:])
            nc.vector.memset(neg_half[:], -0.5)

            # Warm the PE HAM clock gate with cheap junk matmuls while the
            # input pipeline fills, so the real stream starts at 2.4 GHz.
            junk = mask_t[:].bitcast(mybir.dt.float16)  # [P, 16] arbitrary bits
            warm_ps = psum.tile([P, 16], mybir.dt.float32, tag="warm")
            for _ in range(N_WARM):
                nc.tensor.matmul(warm_ps[:16, :], junk, junk, start=True, stop=True)

            _last_tr = [None]

            def load_bytes(bp_d, m, k_blocks):
                kb = k_blocks * P // 8  # bytes per row
                byt = wpipe.tile([P, kb], mybir.dt.int32, tag="bytes", bufs=4)
                nc.sync.dma_start(byt[:], bp_d[m * P : (m + 1) * P, :])
                return byt

            def unpack_quarters(byt, k_blocks, scale_s0):
                """Unpack a loaded 128-row byte block into its transposed
                [P, k_blocks, P] weight tile, quarter by quarter.
                scale_s0: also multiply by the replicated scaling0 row."""
                wT = wpipe.tile([P, k_blocks, P], mybir.dt.float16, tag="wT", bufs=4)
                for c0 in range(0, k_blocks * P, uch):
                    nb = uch // 8
                    b0 = c0 // 8
                    masked = wpipe.tile([P, uch], mybir.dt.int32, tag="masked", bufs=3)
                    in0 = byt[:, b0 : b0 + nb][:, :, None].broadcast_to([P, nb, 8])
                    in1 = mask_t[:][:, None, :].broadcast_to([P, nb, 8])
                    nc.vector.tensor_tensor(
                        masked[:].rearrange("p (b j) -> p b j", j=8),
                        in0,
                        in1,
                        mybir.AluOpType.bitwise_and,
                    )
                    wq = wpipe.tile([P, uch], mybir.dt.float16, tag="wnat", bufs=4)
                    nc.scalar.activation(
                        wq[:], masked[:], Act.Sign, bias=neg_half[:, 0:1]
                    )
                    if scale_s0:
                        nc.vector.tensor_tensor(
                            wq[:], wq[:], s0r_t[:, c0 : c0 + uch],
                            mybir.AluOpType.mult,
                        )
                    _last_tr[0] = nc.sync.dma_start_transpose(
                        wT[:, c0 // P : (c0 + uch) // P, :], wq[:]
                    ).ins
                return wT

            def unpack_wT(bp_d, m, k_blocks, scale_s0):
                return unpack_quarters(load_bytes(bp_d, m, k_blocks), k_blocks, scale_s0)

            # x.T in two batch halves: xH[h][p, k, r] = x[h*b/2 + r, 128k + p].
            # Band-split whole-half transposes read DRAM contiguously; no
            # scaling needed (scaling0 lives in W1).
            half = b // 2
            xH = [
                big.tile([P, in_k, half], mybir.dt.float16, tag=f"xT{h}", name=f"xh{h}")
                for h in range(2)
            ]

            def x_bands(h, after=None):
                # One full-width transpose per half: the DRAM read is fully
                # contiguous (whole rows), and 1024 xbar tiles keeps the DMA
                # semaphore threshold within the ISA field.
                tr = nc.sync.dma_start_transpose(
                    xH[h][:], x_d[h * half : (h + 1) * half, :]
                )
                if after is not None:
                    add_dep_helper(tr.ins, after, reason="x half-2 after startup wT")

            # Startup: prefetch byte blocks, transpose the first x half, then
            # unpack the first START_BLOCKS weight blocks, then the second x
            # half. The PE runs c0 passes of blocks 0..3 against the first x
            # half while the second is still transposing.
            SB = min(4, mid_k)
            byts = [load_bytes(bp1_d, m, in_k) for m in range(SB)]
            x_bands(0)
            wTs = [unpack_quarters(byts[m], in_k, True) for m in range(SB)]

            hT = big.tile([P, mid_k, b], mybir.dt.float16)

            def g1_pass(m, wT, c):
                ps = psum.tile([P, fd], mybir.dt.float32, tag="ps")
                for k in range(in_k):
                    nc.tensor.matmul(
                        ps[:],
                        wT[:, k, :],
                        xH[c][:, k, :],
                        start=(k == 0),
                        stop=(k == in_k - 1),
                    )
                nc.scalar.activation(
                    hT[:, m, c * fd : (c + 1) * fd],
                    ps[:],
                    Act.Copy,
                    scale=s2_t[:, m : m + 1],
                )

            # c-major startup over the first SB blocks; the second x half
            # transposes while the first-half passes run on the PE.
            for m in range(SB):
                g1_pass(m, wTs[m], 0)
            x_bands(1, after=_last_tr[0])
            for c in range(1, nbc):
                for m in range(SB):
                    g1_pass(m, wTs[m], c)

            # Unified steady loop: GEMM1 blocks SB.., then GEMM2 blocks, with
            # weight unpack prefetched two blocks ahead.
            n_blocks = mid_k + out_k

            def mk(jj):
                if jj >= n_blocks:
                    return None
                if jj < mid_k:
                    return unpack_wT(bp1_d, jj, in_k, True)
                return unpack_wT(bp3_d, jj - mid_k, mid_k, False)

            # GEMM2 output staging: groups of blocks buffered in the (dead)
            # x-half SBUF slots, stored with one DMA per group; the final
            # group is kept small so the tail store is short.
            yT_v = yT_d.rearrange("(g p) c -> p g c", p=P)
            ygroups = []
            _o = 0
            while _o < out_k:
                rem = out_k - _o
                if rem > 8:
                    n = 8
                elif rem > 2:
                    n = rem - 2
                else:
                    n = rem
                ygroups.append((_o, n))
                _o += n
            o2group = {}
            for gi_, (gs, gn) in enumerate(ygroups):
                for oo in range(gs, gs + gn):
                    o2group[oo] = (gi_, gs, gn)
            yt_g = None
            pend = [mk(SB), mk(SB + 1)]
            for j in range(SB, n_blocks):
                wT = pend.pop(0)
                pend.append(mk(j + 2))
                if j < mid_k:  # GEMM1 block
                    for c in range(nbc):
                        g1_pass(j, wT, c)
                else:  # GEMM2 block
                    o = j - mid_k
                    gi_, gstart, glen = o2group[o]
                    if o == gstart:
                        yt_g = big.tile(
                            [P, glen, b], mybir.dt.float16,
                            tag=f"xT{gi_ % 2}", name=f"ytg{o}",
                        )
                    for c in range(nbc):
                        ps = psum.tile([P, fd], mybir.dt.float32, tag="ps")
                        for k in range(mid_k):
                            nc.tensor.matmul(
                                ps[:],
                                wT[:, k, :],
                                hT[:, k, c * fd : (c + 1) * fd],
                                start=(k == 0),
                                stop=(k == mid_k - 1),
                            )
                        nc.scalar.activation(
                            yt_g[:, o - gstart, c * fd : (c + 1) * fd],
                            ps[:],
                            Act.Identity,
                            bias=bias_t[:, o : o + 1],
                            scale=s4_t[:, o : o + 1],
                        )
                    if o == gstart + glen - 1:
                        nc.sync.dma_start(
                            yT_v[:, gstart : gstart + glen, :], yt_g[:]
                        )

    nc.compile()
    return nc


def make_in_maps(x, scaling0, bp1, scaling2, bp3, scaling4, bias, n_cores=N_CORES):
    b_full, in_ = x.shape
    mid = scaling2.shape[0]
    out = scaling4.shape[0]
    b = b_full // n_cores

    mask = (1 << (7 - np.arange(8, dtype=np.int32)))[None, :].repeat(P, 0)
    mask = np.ascontiguousarray(mask.astype(np.int32))

    def pcol(v):
        return np.ascontiguousarray(v.astype(np.float32).reshape(-1, P).T)

    shared = {
        "bp1": np.ascontiguousarray(bp1.reshape(mid, in_ // 8)),
        "bp3": np.ascontiguousarray(bp3.reshape(out, mid // 8)),
        "mask": mask,
        "s0rep": np.ascontiguousarray(
            np.broadcast_to(scaling0.astype(np.float16)[None, :], (P, in_))
        ),
        "s2": pcol(scaling2),
        "s4": pcol(scaling4),
        "bias": pcol(bias),
    }
    return [
        {"x": np.ascontiguousarray(x[c * b : (c + 1) * b]), **shared}
        for c in range(n_cores)
    ]


_PROGRAM_CACHE = {}


def run(x, scaling0, bp1, scaling2, bp3, scaling4, bias, **spmd_kwargs):
    """Compile (cached) + run on 8 cores; returns (y, BassKernelResults)."""
    if "nc" not in _PROGRAM_CACHE:
        _PROGRAM_CACHE["nc"] = build_program()
    nc = _PROGRAM_CACHE["nc"]
    in_maps = make_in_maps(x, scaling0, bp1, scaling2, bp3, scaling4, bias)
    res = run_bass_kernel_spmd(nc, in_maps, core_ids=list(range(N_CORES)), **spmd_kwargs)
    b = x.shape[0] // N_CORES
    y = np.empty((x.shape[0], scaling4.shape[0]), dtype=np.float16)
    for c in range(N_CORES):
        y[c * b : (c + 1) * b] = res.results[c]["yT"].T
    return y, res


def kernel(x, scaling0, bp1, scaling2, bp3, scaling4, bias):
    y, _ = run(x, scaling0, bp1, scaling2, bp3, scaling4, bias)
    return y
